# revision 2
# baseline (speedup 1.0000x reference)
"""Trainium2 Bass kernel for nn_BuildLstmUnrollNet (2-layer LSTM, 11-step unroll,
per-step weights), distributed over 8 NeuronCores.

Strategy: 8-way tensor parallelism over the 4*R gate dimension. Each core owns a
128-row slice of each of the 4 gates (512 of 4096 pre-activation columns), so
per-step weight reads are fully disjoint across cores (HBM traffic /8). The
full batch (256 = 2 partition tiles) is kept on every core. Matmuls run with
activations stationary (x^T / h^T tiles) and weight slices moving, in bf16 with
fp32 PSUM accumulation; biases are folded in as K=1 rank-1 matmuls. After each
cell's gating, the new h slice is PE-transposed, cast to bf16 and AllGathered
(2 small collectives per step) to rebuild the replicated h^T stationary tiles.
Cell states c stay core-local in fp32. The scan emits the top-layer h *before*
each step's update, so only 10 of the 11 unrolled steps are computed; the
11th step's weights are dead.

Only the final assembled output is fp32-exact layout-wise; matmul precision is
bf16 (measured end-to-end rel. error ~7e-4 vs the fp32 reference).
"""
import numpy as np

B, I, R, L, U = 256, 512, 1024, 2, 11
U_RUN = U - 1          # the 11th step never reaches the output
N_CORES = 8
RC = 128               # per-core rows per gate
W = 4 * RC             # per-core pre width (512)
NKX = I // 128         # 4  x   k-tiles (layer0 input part)
NKH = R // 128         # 8  h   k-tiles
NK0 = NKX + NKH        # 12 layer0 k-tiles
NK1 = 2 * NKH          # 16 layer1 k-tiles (Wh first, then Wi)
NB = B // 128          # 2 batch tiles


def build_program(reps: int = 1):
    import concourse.bacc as bacc
    import concourse.mybir as mybir
    import concourse.tile as tile
    import concourse.masks as masks

    F32 = mybir.dt.float32
    BF16 = mybir.dt.bfloat16
    Sig = mybir.ActivationFunctionType.Sigmoid
    Tanh = mybir.ActivationFunctionType.Tanh

    nc = bacc.Bacc("TRN2", target_bir_lowering=False, debug=False,
                   num_devices=N_CORES)

    xT_d = nc.dram_tensor("xT", [I, B], BF16, kind="ExternalInput")
    h0T_d = nc.dram_tensor("h0T0", [R, B], BF16, kind="ExternalInput")
    h1T_d = nc.dram_tensor("h1T0", [R, B], BF16, kind="ExternalInput")
    c0_d = nc.dram_tensor("c0i", [B, RC], F32, kind="ExternalInput")
    c1_d = nc.dram_tensor("c1i", [B, RC], F32, kind="ExternalInput")
    w0_d = nc.dram_tensor("w0", [U_RUN, NK0, 128, W], BF16, kind="ExternalInput")
    w1_d = nc.dram_tensor("w1", [U_RUN, NK1, 128, W], BF16, kind="ExternalInput")
    b0_d = nc.dram_tensor("b0", [1, U_RUN * W], BF16, kind="ExternalInput")
    b1_d = nc.dram_tensor("b1", [1, U_RUN * W], BF16, kind="ExternalInput")
    y_d = nc.dram_tensor("y", [U_RUN, B, RC], F32, kind="ExternalOutput")

    with tile.TileContext(nc) as tc:
        with tc.tile_pool(name="const", bufs=1) as constp, \
             tc.tile_pool(name="hpool", bufs=2) as hpool, \
             tc.tile_pool(name="cpool", bufs=2) as cpool, \
             tc.tile_pool(name="wpool", bufs=2) as wpool, \
             tc.tile_pool(name="gpool", bufs=2) as gpool, \
             tc.tile_pool(name="stage", bufs=2) as stage, \
             tc.tile_pool(name="ps0", bufs=2, space="PSUM") as ps0, \
             tc.tile_pool(name="ps1", bufs=1, space="PSUM") as ps1, \
             tc.tile_pool(name="psT", bufs=2, space="PSUM") as psT, \
             tc.tile_pool(name="dramp", bufs=2, space="DRAM") as dramp:

            # ---- constants ----
            ident = constp.tile([128, 128], F32)
            masks.make_identity(nc, ident[:])
            ones = constp.tile([1, 128], BF16)
            nc.gpsimd.memset(ones[:], 1.0)
            xT_s = constp.tile([128, NKX * B], BF16)
            nc.sync.dma_start(
                xT_s[:].rearrange("p (k b) -> p k b", k=NKX),
                xT_d.ap().rearrange("(k p) b -> p k b", p=128),
            )
            bias0 = constp.tile([1, U_RUN * W], BF16)
            nc.sync.dma_start(bias0[:], b0_d.ap())
            bias1 = constp.tile([1, U_RUN * W], BF16)
            nc.sync.dma_start(bias1[:], b1_d.ap())

            # ---- helpers ----
            def load_h(dst_name, src_ap):
                t = hpool.tile([128, NKH * B], BF16, name=dst_name, tag=dst_name)
                nc.sync.dma_start(
                    t[:].rearrange("p (k b) -> p k b", k=NKH),
                    src_ap.rearrange("(k p) b -> p k b", p=128),
                )
                return t

            def load_w(dst_name, src_ap, nk):
                t = wpool.tile([128, nk * W], BF16, name=dst_name, tag=dst_name)
                nc.sync.dma_start(
                    t[:].rearrange("p (k n) -> p k n", k=nk),
                    src_ap.transpose([1, 0, 2]),
                )
                return t

            def hstat(h_s, kk, b):
                return h_s[:, kk * B + b * 128 : kk * B + (b + 1) * 128]

            def gate_cell(layer, b, pre_ps, c_old):
                """LSTM cell elementwise part; returns (c_new, h_new[f32 sbuf])."""
                sig = gpool.tile([128, 3 * RC], F32, name=f"sig{layer}_{b}",
                                 tag=f"sig{layer}_{b}")
                nc.scalar.activation(sig[:], pre_ps[:, : 3 * RC], Sig)
                tg = gpool.tile([128, RC], F32, name=f"tg{layer}_{b}",
                                tag=f"tg{layer}_{b}")
                nc.scalar.activation(tg[:], pre_ps[:, 3 * RC :], Tanh)
                t1 = gpool.tile([128, RC], F32, name=f"t1{layer}_{b}",
                                tag=f"t1{layer}_{b}")
                nc.vector.tensor_mul(t1[:], sig[:, :RC], tg[:])
                t2 = gpool.tile([128, RC], F32, name=f"t2{layer}_{b}",
                                tag=f"t2{layer}_{b}")
                nc.vector.tensor_mul(t2[:], sig[:, RC : 2 * RC], c_old[:])
                cnew = cpool.tile([128, RC], F32, name=f"c{layer}_{b}",
                                  tag=f"c{layer}_{b}")
                nc.vector.tensor_add(cnew[:], t1[:], t2[:])
                tcc = gpool.tile([128, RC], F32, name=f"tc{layer}_{b}",
                                 tag=f"tc{layer}_{b}")
                nc.scalar.activation(tcc[:], cnew[:], Tanh)
                hnew = gpool.tile([128, RC], F32, name=f"h{layer}_{b}",
                                  tag=f"h{layer}_{b}")
                nc.vector.tensor_mul(hnew[:], sig[:, 2 * RC : 3 * RC], tcc[:])
                return cnew, hnew

            def transpose_gather(layer, hnew):
                """PE-transpose both b-tiles of h slice, cast bf16, AllGather,
                return new replicated hT tile."""
                tps = psT.tile([128, B], F32, name=f"tps{layer}", tag="tps")
                for b in range(NB):
                    nc.tensor.transpose(tps[:, b * 128 : (b + 1) * 128],
                                        hnew[b][:], ident[:])
                hst = stage.tile([128, B], BF16, name=f"hst{layer}",
                                 tag=f"hst{layer}")
                nc.scalar.copy(hst[:], tps[:])
                ag_in = dramp.tile([128, B], BF16, name=f"agin{layer}",
                                   tag=f"agin{layer}")
                nc.sync.dma_start(ag_in[:], hst[:])
                ag_out = dramp.tile([R, B], BF16, name=f"agout{layer}",
                                    tag=f"agout{layer}", addr_space="Shared")
                nc.gpsimd.collective_compute(
                    "AllGather", mybir.AluOpType.bypass,
                    replica_groups=[list(range(N_CORES))],
                    ins=[ag_in[:]], outs=[ag_out[:]],
                )
                return load_h(f"h{layer}T_s", ag_out[:])

            def start_pre0(widx, w0_s):
                """Bias + x-part of layer0 pre for weight-step widx."""
                tiles = []
                for b in range(NB):
                    p = ps0.tile([128, W], F32, name=f"pre0_{b}", tag=f"pre0_{b}")
                    nc.tensor.matmul(p[:], ones[:],
                                     bias0[:, widx * W : (widx + 1) * W],
                                     start=True, stop=False)
                    for kk in range(NKX):
                        nc.tensor.matmul(
                            p[:], hstat(xT_s, kk, b),
                            w0_s[:, kk * W : (kk + 1) * W],
                            start=False, stop=False,
                        )
                    tiles.append(p)
                return tiles

            # ---- prologue ----
            h0T_s = load_h("h0T_s", h0T_d.ap())
            h1T_s = load_h("h1T_s", h1T_d.ap())
            c = {}
            for layer, cd in ((0, c0_d), (1, c1_d)):
                for b in range(NB):
                    t = cpool.tile([128, RC], F32, name=f"c{layer}_{b}",
                                   tag=f"c{layer}_{b}")
                    nc.sync.dma_start(t[:], cd.ap()[b * 128 : (b + 1) * 128, :])
                    c[layer, b] = t
            w0_s = load_w("w0_s", w0_d.ap()[0], NK0)
            w1_s = load_w("w1_s", w1_d.ap()[0], NK1)
            pre0_pend = start_pre0(0, w0_s)

            # ---- steps ----
            for rep in range(reps):
                for t in range(U_RUN):
                    widx = t
                    nidx = (t + 1) % U_RUN
                    has_next = not (rep == reps - 1 and t == U_RUN - 1)
                    write_y = rep == 0

                    # (a) prefetch next step's weights
                    if has_next:
                        w0_n = load_w("w0_s", w0_d.ap()[nidx], NK0)
                        w1_n = load_w("w1_s", w1_d.ap()[nidx], NK1)

                    # (b) finish pre0: recurrent part
                    for b in range(NB):
                        p = pre0_pend[b]
                        for kh in range(NKH):
                            nc.tensor.matmul(
                                p[:], hstat(h0T_s, kh, b),
                                w0_s[:, (NKX + kh) * W : (NKX + kh + 1) * W],
                                start=False, stop=(kh == NKH - 1),
                            )

                    # (c) layer0 gating
                    h0new = []
                    for b in range(NB):
                        c[0, b], hn = gate_cell(0, b, pre0_pend[b], c[0, b])
                        h0new.append(hn)

                    # (d) transpose + AllGather h0 slice
                    h0T_s = transpose_gather(0, h0new)

                    # (e) pre1: bias + Wh1 part (old h1)
                    pre1 = []
                    for b in range(NB):
                        p = ps1.tile([128, W], F32, name=f"pre1_{b}",
                                     tag=f"pre1_{b}")
                        nc.tensor.matmul(p[:], ones[:],
                                         bias1[:, widx * W : (widx + 1) * W],
                                         start=True, stop=False)
                        for kh in range(NKH):
                            nc.tensor.matmul(
                                p[:], hstat(h1T_s, kh, b),
                                w1_s[:, kh * W : (kh + 1) * W],
                                start=False, stop=False,
                            )
                        pre1.append(p)

                    # (f) early start of next step's pre0 (x part) — fills the
                    # PE while the h0 AllGather is in flight
                    if has_next:
                        pre0_pend = start_pre0(nidx, w0_n)

                    # (g) pre1: Wi1 part (new h0, post-gather)
                    for b in range(NB):
                        for kh in range(NKH):
                            nc.tensor.matmul(
                                pre1[b][:], hstat(h0T_s, kh, b),
                                w1_s[:, (NKH + kh) * W : (NKH + kh + 1) * W],
                                start=False, stop=(kh == NKH - 1),
                            )

                    # (h) layer1 gating (+ output write)
                    h1new = []
                    for b in range(NB):
                        c[1, b], hn = gate_cell(1, b, pre1[b], c[1, b])
                        h1new.append(hn)
                        if write_y:
                            nc.sync.dma_start(
                                y_d.ap()[t, b * 128 : (b + 1) * 128, :], hn[:])

                    # (i) transpose + AllGather h1 slice
                    h1T_s = transpose_gather(1, h1new)

                    if has_next:
                        w0_s, w1_s = w0_n, w1_n

    nc.compile()
    return nc


def prepare_in_maps(inputs: dict) -> list[dict]:
    import ml_dtypes
    bf = ml_dtypes.bfloat16

    x = np.ascontiguousarray(np.asarray(inputs["x"], np.float32))
    st = np.asarray(inputs["init_states_input"], np.float32).reshape(B, 2 * L, R)
    h0i, c0i, h1i, c1i = st[:, 0], st[:, 1], st[:, 2], st[:, 3]

    xT = x.T.astype(bf)
    h0T = h0i.T.astype(bf)
    h1T = h1i.T.astype(bf)

    Wi0 = np.asarray(inputs["Wi0"], np.float32)[:U_RUN]
    Wh0 = np.asarray(inputs["Wh0"], np.float32)[:U_RUN]
    Wi1 = np.asarray(inputs["Wi1"], np.float32)[:U_RUN]
    Wh1 = np.asarray(inputs["Wh1"], np.float32)[:U_RUN]
    b0_full = (np.asarray(inputs["bi0"], np.float32)
               + np.asarray(inputs["bh0"], np.float32))[:U_RUN]
    b1_full = (np.asarray(inputs["bi1"], np.float32)
               + np.asarray(inputs["bh1"], np.float32))[:U_RUN]

    in_maps = []
    for k in range(N_CORES):
        rows = np.concatenate(
            [np.arange(g * R + k * RC, g * R + (k + 1) * RC) for g in range(4)])
        # layer0 moving weights: [U, I+R, W] -> [U, NK0, 128, W]
        w0 = np.concatenate(
            [Wi0[:, rows, :].transpose(0, 2, 1),
             Wh0[:, rows, :].transpose(0, 2, 1)], axis=1)
        w0 = np.ascontiguousarray(w0.reshape(U_RUN, NK0, 128, W)).astype(bf)
        # layer1: Wh part first, then Wi part
        w1 = np.concatenate(
            [Wh1[:, rows, :].transpose(0, 2, 1),
             Wi1[:, rows, :].transpose(0, 2, 1)], axis=1)
        w1 = np.ascontiguousarray(w1.reshape(U_RUN, NK1, 128, W)).astype(bf)
        in_maps.append({
            "xT": xT,
            "h0T0": h0T,
            "h1T0": h1T,
            "c0i": np.ascontiguousarray(c0i[:, k * RC : (k + 1) * RC]),
            "c1i": np.ascontiguousarray(c1i[:, k * RC : (k + 1) * RC]),
            "w0": w0,
            "w1": w1,
            "b0": np.ascontiguousarray(b0_full[:, rows].reshape(1, -1)).astype(bf),
            "b1": np.ascontiguousarray(b1_full[:, rows].reshape(1, -1)).astype(bf),
        })
    return in_maps


def assemble_output(inputs: dict, results: list[dict]) -> np.ndarray:
    st = np.asarray(inputs["init_states_input"], np.float32).reshape(B, 2 * L, R)
    h1i = st[:, 2]
    out = np.empty((B, U * R), np.float32)
    out[:, :R] = h1i
    for k in range(N_CORES):
        y = results[k]["y"]  # [U_RUN, B, RC]
        for s in range(U_RUN):
            out[:, (s + 1) * R + k * RC : (s + 1) * R + (k + 1) * RC] = y[s]
    return out


_CACHE: dict = {}


def _get_compiled():
    if "nc" not in _CACHE:
        _CACHE["nc"] = build_program(reps=1)
    return _CACHE["nc"]


def kernel(**inputs) -> np.ndarray:
    from concourse.bass_utils import run_bass_kernel_spmd

    nc = _get_compiled()
    in_maps = prepare_in_maps(inputs)
    res = run_bass_kernel_spmd(nc, in_maps, list(range(N_CORES)))
    return assemble_output(inputs, res.results)
